# revision 7
# baseline (speedup 1.0000x reference)
"""Trainium2 Bass kernel for nn_LoopWithIf.

The reference loop
    for i in range(32):
        b = 3*a; s = sum(b); a = a+b if s>0 else a-b
collapses algebraically: the gate's sign is deterministic after the first
iteration, and scaling by 4 / -2 is exact in fp32 (powers of two), so
    out = inp * 2**64      if sum(inp) > 0
    out = inp * -(2**63)   otherwise

Kernel structure (single NEFF, SPMD over 8 NeuronCores, 16MB/core kept
SBUF-resident so the data is read from HBM exactly once):

  phase 1   pipelined DMA loads of contiguous slabs (7x2MB + 1MB + 0.5MB +
            0.5MB) + per-chunk reduce_sum on DVE; tapering chunk sizes
            shorten the reduce tail after the last byte lands.

  gate      NO ncfw collective on the critical path (collective_compute
            costs ~50us of fixed dispatch latency in this runtime).
            Instead: an XOR-pattern cross-core gather with raw SDMA
            remote_dma_broadcast:
              - the local total is broadcast to all partitions via a
                ones-matmul
              - core s sends its [128,1] total to peer s^d (d=1..7),
                writing column d of a [128,8] SBUF tile; every address is
                compile-time (receiver r's column d holds core r^d's total,
                modulo a fixed cross-die permutation the sum ignores)
              - descriptors are prepared during the load phase (SWDGE
                prepare-only) and fired by one trigger_dma gated on the
                local total; each arrival bumps a hardware semaphore by 2
              - the consuming reduce carries `._wait_ge(sem, 14)` inside a
                tile_critical block (outside one, the schedule-time sim
                deadlocks on the cross-core semaphore). ONLY the reduce
                lives in the critical: within tile_critical Tile does not
                insert same-engine RAW semaphores, so any dependent op
                there reads stale data.
            The factor select (two DVE tensor_scalar ops) runs outside.

  launch    A dummy AllGather on junk data makes the runtime do a
            synchronized cross-core launch (without any collective in the
            NEFF, launch skew was measured at up to ~8ms). It is forced
            AFTER the gate via a sync dep so its ~40us dispatch parks
            gpsimd only during the store phase. Its data is never read.

  phase 2   in-place scale by the factor (DVE, exact power-of-two multiply)
            + pipelined stores, smallest chunk first so the store pipeline
            starts right after the factor resolves.

Runtime branching (tc.If / value_load) crashes or fails codegen under
this PJRT/axon execution path, so the kernel is straight-line; the
factor select is pure data flow.
"""

import numpy as np

N_CORES = 8
ROWS = 32            # inp.shape[0]
ROWS_PER_CORE = ROWS // N_CORES
P = 128              # SBUF partitions
TOTAL = ROWS_PER_CORE * 1024 * 1024          # 4194304 elements per core

# Wider chunks -> longer per-partition DMA lines -> fewer, larger
# descriptors. The HWDGE queues dispatch ~1 descriptor per ~100ns, so at
# 16KB lines dispatch (not engine bandwidth) limits throughput; 32KB lines
# halve the descriptor count. Tapered tail keeps the post-load reduce short.
LOAD_W = [8192, 8192, 8192, 4096, 2048, 1024, 512, 512]  # sum = 32768
# big chunks first so the DMA queue is deep from the start; small chunks
# last so the reduce tail after the final byte is ~1us. Stores go smallest
# first so the store pipeline starts right after the factor resolves.
STORE_ORDER = [7, 6, 5, 4, 3, 0, 1, 2]

_nc = None  # compiled kernel cache


def _build(n_cores=N_CORES):
    import concourse.bass as bass  # noqa: F401
    import concourse.bacc as bacc
    import concourse.mybir as mybir
    import concourse.tile as tile
    from concourse.tile_rust import add_dep_helper

    f32 = mybir.dt.float32
    nc = bacc.Bacc(
        "TRN2",
        target_bir_lowering=False,
        debug=False,
        enable_asserts=False,
        num_devices=n_cores,
    )
    inp_d = nc.dram_tensor("inp", [TOTAL], f32, kind="ExternalInput").ap()
    out_d = nc.dram_tensor("out", [TOTAL], f32, kind="ExternalOutput").ap()

    # chunk i occupies the contiguous slab [off_i, off_i + P*w_i) viewed as
    # [P, w_i] (partition-major) -- single-descriptor-friendly like the
    # baseline's slab layout
    offs = np.concatenate([[0], np.cumsum([P * w for w in LOAD_W])]).tolist()
    nch = len(LOAD_W)

    def slab(dram, i):
        w = LOAD_W[i]
        return dram[offs[i] : offs[i] + P * w].rearrange("(p w) -> p w", p=P)

    with tile.TileContext(nc) as tc:
        with (
            tc.tile_pool(name="data", bufs=1) as data_pool,
            tc.tile_pool(name="small", bufs=1) as small_pool,
            tc.tile_pool(name="psum", bufs=1, space="PSUM") as psum_pool,
            tc.tile_pool(name="dram", bufs=1, space="DRAM") as dram_pool,
        ):
            # Semaphores start at 0 each execution (same runtime guarantee
            # Tile's own DMA sems rely on); do NOT sem_clear here -- a
            # scheduler-placed clear can race peer arrivals and wipe them.
            gather_sem = nc.alloc_semaphore("xgather")
            send_sem = nc.alloc_semaphore("xsend")

            chunks = [
                data_pool.tile([P, w], f32, name=f"xchunk{i}", tag=f"xchunk{i}")
                for i, w in enumerate(LOAD_W)
            ]
            partials = small_pool.tile([P, nch], f32, name="partials")
            ones = small_pool.tile([P, P], f32, name="ones")
            nc.vector.memset(ones[:], 1.0)
            buf = small_pool.tile([P, n_cores], f32, name="xbuf")
            tloc = small_pool.tile([P, 1], f32, name="tloc")
            # Early producer for tloc so the gather preps (emitted below,
            # before the real total exists) schedule during the load phase;
            # the trigger's deferred RAW dep binds to the LAST writer (the
            # real copy), keeping the data race-free.
            nc.vector.memset(tloc[:], 0.0)

            # XOR-gather descriptor preps: Q7 desc-gen is ~0.8us each, so
            # emit them here to run under the loads; tloc is only read when
            # trigger_dma fires.
            for d in range(1, n_cores):
                rdests = [None] * n_cores
                rdests[d] = (0, d)
                nc.gpsimd.remote_dma_broadcast(
                    out_ap=buf[:, d : d + 1],
                    in_ap=tloc[:],
                    remote_sem=gather_sem,
                    local_sem=send_sem,
                    rdests=rdests,
                )

            # phase 1: pipelined load + per-chunk reduce; alternate the two
            # HWDGE rings (sync / scalar) so consecutive transfers' fixed
            # costs (~2us completion receipt each) overlap
            with nc.named_scope("load"):
                for i in range(nch):
                    eng = nc.sync if i % 2 == 0 else nc.scalar
                    eng.dma_start(chunks[i][:], slab(inp_d, i))
                    nc.vector.reduce_sum(
                        partials[:, i : i + 1], chunks[i][:], axis=mybir.AxisListType.X
                    )


            # local total, broadcast to all 128 partitions via ones-matmul
            sc_gate = nc.named_scope("gate")
            sc_gate.__enter__()
            plocal = small_pool.tile([P, 1], f32, name="plocal")
            nc.vector.reduce_sum(plocal[:], partials[:], axis=mybir.AxisListType.X)
            tpsum = psum_pool.tile([P, 1], f32, name="tpsum")
            nc.tensor.matmul(tpsum[:], ones[:], plocal[:])
            tlc = nc.vector.tensor_copy(tloc[:], tpsum[:])
            nc.vector.tensor_copy(buf[:, 0:1], tloc[:])  # self column

            # The preps' deferred RAW binds to tloc's writer at PREP trace
            # time (the memset -- measured: without this explicit edge the
            # trigger fires after the memset and broadcasts zeros), so gate
            # the trigger on the REAL total explicitly.
            trig = nc.gpsimd.trigger_dma(count=None)
            add_dep_helper(
                trig.ins, tlc.ins, True, "fire gather only after real total"
            )

            # gate: wait for the 7 peer arrivals (2 sem incs each), total
            gtot = small_pool.tile([P, 1], f32, name="gtot")
            fac = small_pool.tile([P, 1], f32, name="fac")
            with tc.tile_critical():
                nc.vector.reduce_sum(
                    gtot[:], buf[:], axis=mybir.AxisListType.X
                )._wait_ge(gather_sem, 2 * (n_cores - 1))

            # factor = 1[tot>0] * 3*2^63 - 2^63  ->  2**64 or -(2**63) (exact)
            isgt = nc.vector.tensor_scalar(
                fac[:], gtot[:], 0.0, None, mybir.AluOpType.is_gt
            )
            nc.vector.tensor_scalar(
                fac[:],
                fac[:],
                float(3 * 2**63),
                float(-(2**63)),
                mybir.AluOpType.mult,
                mybir.AluOpType.add,
            )

            sc_gate.__exit__(None, None, None)

            # dummy collective for synchronized launch, AFTER the gate so
            # its dispatch parks gpsimd only during the store phase
            dcc_in = dram_pool.tile([P, 1], f32, name="dcc_in")
            dcc_out = dram_pool.tile(
                [n_cores * P, 1], f32, name="dcc_out", addr_space="Shared"
            )
            dcc = nc.gpsimd.collective_compute(
                "AllGather",
                mybir.AluOpType.bypass,
                replica_groups=[list(range(n_cores))],
                ins=[dcc_in.opt()],
                outs=[dcc_out.opt()],
            )
            # add_dep_helper(A, B) = A waits on B -- dcc waits on the gate
            add_dep_helper(
                dcc.ins, isgt.ins, True, "dummy cc after gate resolves"
            )

            # phase 2: in-place scale (DVE) + store, smallest chunk first,
            # alternating HWDGE rings like the loads
            with nc.named_scope("store"):
                for k, i in enumerate(STORE_ORDER):
                    nc.vector.tensor_scalar_mul(chunks[i][:], chunks[i][:], fac[:])
                    eng = nc.sync if k % 2 == 0 else nc.scalar
                    eng.dma_start(slab(out_d, i), chunks[i][:])

    nc.compile()
    return nc


def _run(in_maps, trace=False):
    from concourse.bass_utils import run_bass_kernel_spmd

    global _nc
    if _nc is None:
        _nc = _build()
    return run_bass_kernel_spmd(
        _nc, in_maps, core_ids=list(range(N_CORES)), trace=trace
    )


def _shard(inp):
    return [
        np.ascontiguousarray(
            inp[c * ROWS_PER_CORE : (c + 1) * ROWS_PER_CORE]
        ).reshape(TOTAL)
        for c in range(N_CORES)
    ]


def _unshard(results):
    out = np.empty((ROWS, 1024, 1024), dtype=np.float32)
    for c in range(N_CORES):
        out[c * ROWS_PER_CORE : (c + 1) * ROWS_PER_CORE] = results[c]["out"].reshape(
            ROWS_PER_CORE, 1024, 1024
        )
    return out


def kernel(**inputs):
    inp = np.ascontiguousarray(np.asarray(inputs["inp"], dtype=np.float32))
    res = _run([{"inp": s} for s in _shard(inp)], trace=False)
    return _unshard(res.results)


def run_traced(inputs):
    """Like kernel() but with NTFF profiling; returns (out, res)."""
    inp = np.ascontiguousarray(np.asarray(inputs["inp"], dtype=np.float32))
    res = _run([{"inp": s} for s in _shard(inp)], trace=True)
    return _unshard(res.results), res



# revision 13
# speedup vs baseline: 1.0413x; 1.0413x over previous
"""Trainium2 Bass kernel for nn_LoopWithIf.

The reference loop
    for i in range(32):
        b = 3*a; s = sum(b); a = a+b if s>0 else a-b
collapses algebraically: the gate's sign is deterministic after the first
iteration, and scaling by 4 / -2 is exact in fp32 (powers of two), so
    out = inp * 2**64      if sum(inp) > 0
    out = inp * -(2**63)   otherwise

Kernel structure (single NEFF, SPMD over 8 NeuronCores, 16MB/core kept
SBUF-resident so the data is read from HBM exactly once):

  phase 1   pipelined DMA loads of contiguous slabs (7x2MB + 1MB + 0.5MB +
            0.5MB) + per-chunk reduce_sum on DVE; tapering chunk sizes
            shorten the reduce tail after the last byte lands.

  gate      NO ncfw collective on the critical path (collective_compute
            costs ~50us of fixed dispatch latency in this runtime).
            Instead: an XOR-pattern cross-core gather with raw SDMA
            remote_dma_broadcast:
              - the local total is broadcast to all partitions via a
                ones-matmul
              - core s sends its [128,1] total to peer s^d (d=1..7),
                writing column d of a [128,8] SBUF tile; every address is
                compile-time (receiver r's column d holds core r^d's total,
                modulo a fixed cross-die permutation the sum ignores)
              - descriptors are prepared during the load phase (SWDGE
                prepare-only) and fired by one trigger_dma gated on the
                local total; each arrival bumps a hardware semaphore by 2
              - the consuming reduce carries `._wait_ge(sem, 14)` inside a
                tile_critical block (outside one, the schedule-time sim
                deadlocks on the cross-core semaphore). ONLY the reduce
                lives in the critical: within tile_critical Tile does not
                insert same-engine RAW semaphores, so any dependent op
                there reads stale data.
            The factor select (two DVE tensor_scalar ops) runs outside.

  launch    A dummy AllGather on junk data makes the runtime do a
            synchronized cross-core launch (without any collective in the
            NEFF, launch skew was measured at up to ~8ms). It is forced
            AFTER the gate via a sync dep so its ~40us dispatch parks
            gpsimd only during the store phase. Its data is never read.

  phase 2   in-place scale by the factor (DVE, exact power-of-two multiply)
            + pipelined stores, smallest chunk first so the store pipeline
            starts right after the factor resolves.

Runtime branching (tc.If / value_load) crashes or fails codegen under
this PJRT/axon execution path, so the kernel is straight-line; the
factor select is pure data flow.
"""

import numpy as np

N_CORES = 8
ROWS = 32            # inp.shape[0]
ROWS_PER_CORE = ROWS // N_CORES
P = 128              # SBUF partitions
TOTAL = ROWS_PER_CORE * 1024 * 1024          # 4194304 elements per core

# Chunk sizing: the DVE reduce consumes ~480GB/s, faster than the ~330GB/s
# DMA arrival rate, so with medium chunks the reduce tracks the load stream
# and the local total is ready ~1us after the last byte. Big chunks in the
# middle keep descriptor count low; small chunks at both ends shorten the
# pipeline ramp and the reduce tail.
LOAD_W = [2048, 2048, 2048, 2048, 4096, 4096, 4096, 4096, 4096,
          2048, 1024, 512, 256, 256]  # sum = 32768
# Stores go smallest first so the store pipeline starts right after the
# factor resolves.
STORE_ORDER = [13, 12, 11, 10, 9, 0, 1, 2, 3, 4, 5, 6, 7, 8]
N_SWDGE_Q = 4  # spread the 7 gather broadcasts over 4 SWDGE rings

_nc = None  # compiled kernel cache


def _build(n_cores=N_CORES):
    import concourse.bass as bass  # noqa: F401
    import concourse.bacc as bacc
    import concourse.mybir as mybir
    import concourse.tile as tile
    from concourse.tile_rust import add_dep_helper

    f32 = mybir.dt.float32
    nc = bacc.Bacc(
        "TRN2",
        target_bir_lowering=False,
        debug=False,
        enable_asserts=False,
        num_devices=n_cores,
        num_swdge_queues=N_SWDGE_Q,
    )
    inp_d = nc.dram_tensor("inp", [TOTAL], f32, kind="ExternalInput").ap()
    out_d = nc.dram_tensor("out", [TOTAL], f32, kind="ExternalOutput").ap()

    # chunk i occupies the contiguous slab [off_i, off_i + P*w_i) viewed as
    # [P, w_i] (partition-major) -- single-descriptor-friendly like the
    # baseline's slab layout
    offs = np.concatenate([[0], np.cumsum([P * w for w in LOAD_W])]).tolist()
    nch = len(LOAD_W)

    def slab(dram, i):
        w = LOAD_W[i]
        return dram[offs[i] : offs[i] + P * w].rearrange("(p w) -> p w", p=P)

    with tile.TileContext(nc) as tc:
        with (
            tc.tile_pool(name="data", bufs=1) as data_pool,
            tc.tile_pool(name="small", bufs=1) as small_pool,
            tc.tile_pool(name="psum", bufs=1, space="PSUM") as psum_pool,
            tc.tile_pool(name="dram", bufs=1, space="DRAM") as dram_pool,
        ):
            # Semaphores start at 0 each execution (same runtime guarantee
            # Tile's own DMA sems rely on); do NOT sem_clear here -- a
            # scheduler-placed clear can race peer arrivals and wipe them.
            gather_sem = nc.alloc_semaphore("xgather")
            send_sem = nc.alloc_semaphore("xsend")

            chunks = [
                data_pool.tile([P, w], f32, name=f"xchunk{i}", tag=f"xchunk{i}")
                for i, w in enumerate(LOAD_W)
            ]
            partials = small_pool.tile([P, nch], f32, name="partials")
            ones = small_pool.tile([P, P], f32, name="ones")
            nc.vector.memset(ones[:], 1.0)
            buf = small_pool.tile([P, n_cores], f32, name="xbuf")
            tloc = small_pool.tile([P, 1], f32, name="tloc")
            # Early producer for tloc so the gather preps (emitted below,
            # before the real total exists) schedule during the load phase;
            # the trigger's deferred RAW dep binds to the LAST writer (the
            # real copy), keeping the data race-free.
            nc.vector.memset(tloc[:], 0.0)

            # XOR-gather descriptor preps: Q7 desc-gen is ~0.8us each, so
            # emit them here to run under the loads; tloc is only read when
            # trigger_dma fires. Each broadcast pushes ~1K descriptors (the
            # ucode emits full 64-desc streams even for dummy lanes) and the
            # ring feeds ~1 desc per ~90ns per lane, so a single ring
            # serializes the 7 sends into ~42us; round-robin over 4 SWDGE
            # rings cuts delivery to ~2 rounds.
            for d in range(1, n_cores):
                rdests = [None] * n_cores
                rdests[d] = (0, d)
                nc.gpsimd.remote_dma_broadcast(
                    out_ap=buf[:, d : d + 1],
                    in_ap=tloc[:],
                    remote_sem=gather_sem,
                    local_sem=send_sem,
                    rdests=rdests,
                    queue_num=(d - 1) % N_SWDGE_Q,
                )

            # phase 1: pipelined load + per-chunk reduce; alternate the two
            # HWDGE rings (sync / scalar) so consecutive transfers' fixed
            # costs (~2us completion receipt each) overlap
            with nc.named_scope("load"):
                for i in range(nch):
                    eng = nc.sync if i % 2 == 0 else nc.scalar
                    eng.dma_start(chunks[i][:], slab(inp_d, i))
                    nc.vector.reduce_sum(
                        partials[:, i : i + 1], chunks[i][:], axis=mybir.AxisListType.X
                    )


            # local total, broadcast to all 128 partitions via ones-matmul
            sc_gate = nc.named_scope("gate")
            sc_gate.__enter__()
            plocal = small_pool.tile([P, 1], f32, name="plocal")
            nc.vector.reduce_sum(plocal[:], partials[:], axis=mybir.AxisListType.X)
            tpsum = psum_pool.tile([P, 1], f32, name="tpsum")
            nc.tensor.matmul(tpsum[:], ones[:], plocal[:])
            tlc = nc.vector.tensor_copy(tloc[:], tpsum[:])
            nc.vector.tensor_copy(buf[:, 0:1], tloc[:])  # self column

            # The preps' deferred RAW binds to tloc's writer at PREP trace
            # time (the memset -- measured: without this explicit edge the
            # trigger fires after the memset and broadcasts zeros), so gate
            # each queue's trigger on the REAL total explicitly.
            for q in range(N_SWDGE_Q):
                trig = nc.gpsimd.trigger_dma(count=None, queue_num=q)
                add_dep_helper(
                    trig.ins, tlc.ins, True, "fire gather only after real total"
                )

            # gate: wait for the 7 peer arrivals (2 sem incs each), total
            gtot = small_pool.tile([P, 1], f32, name="gtot")
            fac = small_pool.tile([P, 1], f32, name="fac")
            with tc.tile_critical():
                nc.vector.reduce_sum(
                    gtot[:], buf[:], axis=mybir.AxisListType.X
                )._wait_ge(gather_sem, 2 * (n_cores - 1))

            # factor = 1[tot>0] * 3*2^63 - 2^63  ->  2**64 or -(2**63) (exact)
            isgt = nc.vector.tensor_scalar(
                fac[:], gtot[:], 0.0, None, mybir.AluOpType.is_gt
            )
            nc.vector.tensor_scalar(
                fac[:],
                fac[:],
                float(3 * 2**63),
                float(-(2**63)),
                mybir.AluOpType.mult,
                mybir.AluOpType.add,
            )

            sc_gate.__exit__(None, None, None)

            # dummy collective for synchronized launch, AFTER the gate so
            # its dispatch parks gpsimd only during the store phase
            dcc_in = dram_pool.tile([P, 1], f32, name="dcc_in")
            dcc_out = dram_pool.tile(
                [n_cores * P, 1], f32, name="dcc_out", addr_space="Shared"
            )
            dcc = nc.gpsimd.collective_compute(
                "AllGather",
                mybir.AluOpType.bypass,
                replica_groups=[list(range(n_cores))],
                ins=[dcc_in.opt()],
                outs=[dcc_out.opt()],
            )
            # add_dep_helper(A, B) = A waits on B -- dcc waits on the gate
            add_dep_helper(
                dcc.ins, isgt.ins, True, "dummy cc after gate resolves"
            )

            # phase 2: in-place scale (DVE) + store, smallest chunk first,
            # alternating HWDGE rings like the loads
            with nc.named_scope("store"):
                for k, i in enumerate(STORE_ORDER):
                    nc.vector.tensor_scalar_mul(chunks[i][:], chunks[i][:], fac[:])
                    eng = nc.sync if k % 2 == 0 else nc.scalar
                    eng.dma_start(slab(out_d, i), chunks[i][:])

    nc.compile()
    return nc


def _run(in_maps, trace=False, trace_cores=None):
    from concourse.bass_utils import run_bass_kernel_spmd

    global _nc
    if _nc is None:
        _nc = _build()
    kw = {}
    if trace_cores is not None:
        kw["trace_cores"] = trace_cores
    return run_bass_kernel_spmd(
        _nc, in_maps, core_ids=list(range(N_CORES)), trace=trace, **kw
    )


def _shard(inp):
    return [
        np.ascontiguousarray(
            inp[c * ROWS_PER_CORE : (c + 1) * ROWS_PER_CORE]
        ).reshape(TOTAL)
        for c in range(N_CORES)
    ]


def _unshard(results):
    out = np.empty((ROWS, 1024, 1024), dtype=np.float32)
    for c in range(N_CORES):
        out[c * ROWS_PER_CORE : (c + 1) * ROWS_PER_CORE] = results[c]["out"].reshape(
            ROWS_PER_CORE, 1024, 1024
        )
    return out


def kernel(**inputs):
    inp = np.ascontiguousarray(np.asarray(inputs["inp"], dtype=np.float32))
    res = _run([{"inp": s} for s in _shard(inp)], trace=False)
    return _unshard(res.results)


def run_traced(inputs, trace_cores=None):
    """Like kernel() but with NTFF profiling; returns (out, res)."""
    inp = np.ascontiguousarray(np.asarray(inputs["inp"], dtype=np.float32))
    res = _run([{"inp": s} for s in _shard(inp)], trace=True, trace_cores=trace_cores)
    return _unshard(res.results), res



# revision 19
# speedup vs baseline: 1.2197x; 1.1714x over previous
"""Trainium2 Bass kernel for nn_LoopWithIf.

The reference loop
    for i in range(32):
        b = 3*a; s = sum(b); a = a+b if s>0 else a-b
collapses algebraically: the gate's sign is deterministic after the first
iteration, and scaling by 4 / -2 is exact in fp32 (powers of two), so
    out = inp * 2**64      if sum(inp) > 0
    out = inp * -(2**63)   otherwise

Kernel structure (single NEFF, SPMD over 8 NeuronCores, 16MB/core kept
SBUF-resident so the data is read from HBM exactly once):

  phase 1   pipelined DMA loads of contiguous slabs (7x2MB + 1MB + 0.5MB +
            0.5MB) + per-chunk reduce_sum on DVE; tapering chunk sizes
            shorten the reduce tail after the last byte lands.

  gate      NO ncfw collective on the critical path (collective_compute
            costs ~50us of fixed dispatch latency in this runtime).
            Instead: an XOR-pattern cross-core gather with raw SDMA
            remote_dma_broadcast:
              - the local total is broadcast to all partitions via a
                ones-matmul
              - core s sends its [128,1] total to peer s^d (d=1..7),
                writing column d of a [128,8] SBUF tile; every address is
                compile-time (receiver r's column d holds core r^d's total,
                modulo a fixed cross-die permutation the sum ignores)
              - descriptors are prepared during the load phase (SWDGE
                prepare-only) and fired by one trigger_dma gated on the
                local total; each arrival bumps a hardware semaphore by 2
              - the consuming reduce carries `._wait_ge(sem, 14)` inside a
                tile_critical block (outside one, the schedule-time sim
                deadlocks on the cross-core semaphore). ONLY the reduce
                lives in the critical: within tile_critical Tile does not
                insert same-engine RAW semaphores, so any dependent op
                there reads stale data.
            The factor select (two DVE tensor_scalar ops) runs outside.

  launch    A dummy AllGather on junk data makes the runtime do a
            synchronized cross-core launch (without any collective in the
            NEFF, launch skew was measured at up to ~8ms). It is forced
            AFTER the gate via a sync dep so its ~40us dispatch parks
            gpsimd only during the store phase. Its data is never read.

  phase 2   in-place scale by the factor (DVE, exact power-of-two multiply)
            + pipelined stores, smallest chunk first so the store pipeline
            starts right after the factor resolves.

Runtime branching (tc.If / value_load) crashes or fails codegen under
this PJRT/axon execution path, so the kernel is straight-line; the
factor select is pure data flow.
"""

import numpy as np

N_CORES = 8
ROWS = 32            # inp.shape[0]
ROWS_PER_CORE = ROWS // N_CORES
P = 128              # SBUF partitions
TOTAL = ROWS_PER_CORE * 1024 * 1024          # 4194304 elements per core

# Chunk sizing: the DVE reduce consumes ~480GB/s, faster than the ~330GB/s
# DMA arrival rate, so with uniform medium chunks the reduce tracks the
# load stream chunk-by-chunk and the local total is ready ~1.5us after the
# last byte lands. Tapered tail shrinks the final reduce.
LOAD_W = [2048] * 15 + [1024, 512, 256, 128, 128]  # sum = 32768, n=20
# Stores go smallest first so the store pipeline starts right after the
# factor resolves.
STORE_ORDER = [19, 18, 17, 16, 15] + list(range(15))
N_SWDGE_Q = 3  # one SWDGE ring per butterfly stage

_nc = None  # compiled kernel cache


def _build(n_cores=N_CORES):
    import concourse.bass as bass  # noqa: F401
    import concourse.bacc as bacc
    import concourse.mybir as mybir
    import concourse.tile as tile
    from concourse.tile_rust import add_dep_helper

    f32 = mybir.dt.float32
    nc = bacc.Bacc(
        "TRN2",
        target_bir_lowering=False,
        debug=False,
        enable_asserts=False,
        num_devices=n_cores,
        num_swdge_queues=N_SWDGE_Q,
    )
    inp_d = nc.dram_tensor("inp", [TOTAL], f32, kind="ExternalInput").ap()
    out_d = nc.dram_tensor("out", [TOTAL], f32, kind="ExternalOutput").ap()

    # chunk i occupies the contiguous slab [off_i, off_i + P*w_i) viewed as
    # [P, w_i] (partition-major) -- single-descriptor-friendly like the
    # baseline's slab layout
    offs = np.concatenate([[0], np.cumsum([P * w for w in LOAD_W])]).tolist()
    nch = len(LOAD_W)

    def slab(dram, i):
        w = LOAD_W[i]
        return dram[offs[i] : offs[i] + P * w].rearrange("(p w) -> p w", p=P)

    with tile.TileContext(nc) as tc:
        with (
            tc.tile_pool(name="data", bufs=1) as data_pool,
            tc.tile_pool(name="small", bufs=1) as small_pool,
            tc.tile_pool(name="psum", bufs=1, space="PSUM") as psum_pool,
            tc.tile_pool(name="dram", bufs=1, space="DRAM") as dram_pool,
        ):
            # Semaphores start at 0 each execution (same runtime guarantee
            # Tile's own DMA sems rely on); do NOT sem_clear here -- a
            # scheduler-placed clear can race peer arrivals and wipe them.
            gather_sem = nc.alloc_semaphore("xgather")
            send_sem = nc.alloc_semaphore("xsend")

            chunks = [
                data_pool.tile([P, w], f32, name=f"xchunk{i}", tag=f"xchunk{i}")
                for i, w in enumerate(LOAD_W)
            ]
            partials = small_pool.tile([P, nch], f32, name="partials")
            ones = small_pool.tile([P, P], f32, name="ones")
            nc.vector.memset(ones[:], 1.0)
            # Butterfly gather state: buf column k receives the stage-k
            # arrival; acc2/acc3 hold the pair/quad sums we forward.
            buf = small_pool.tile([P, 3], f32, name="xbuf")
            tloc = small_pool.tile([P, 1], f32, name="tloc")
            acc2 = small_pool.tile([P, 1], f32, name="acc2")
            acc3 = small_pool.tile([P, 1], f32, name="acc3")
            # Send-source tiles, written OUTSIDE the criticals: the triggers
            # must dep on an out-of-critical instruction (deps into a
            # critical's inner bb never resolve in the schedule sim).
            s2src = small_pool.tile([P, 1], f32, name="s2src")
            s3src = small_pool.tile([P, 1], f32, name="s3src")
            # Early producers so the gather preps (emitted below, before the
            # real values exist) schedule during the load phase; each
            # trigger's deferred RAW dep binds explicitly to the real writer.
            nc.vector.memset(tloc[:], 0.0)
            nc.vector.memset(s2src[:], 0.0)
            nc.vector.memset(s3src[:], 0.0)

            # Recursive-doubling (XOR butterfly) gather: 3 broadcast stages
            # (d = 1, 2, 4) instead of 7 one-shot sends. Each
            # remote_dma_broadcast pushes ~1056 descriptors through the SDMA
            # rings regardless of payload (the ucode emits full 64-desc
            # streams even for dummy lanes) and the global descriptor feed
            # sustains ~1 desc/6ns, so delivery cost is ~6.3us per
            # *broadcast*: 3 stages ~20us vs ~44us for 7 sends. Stage k
            # lives on its own SWDGE ring so its trigger fires only its own
            # descriptors. XOR pairing makes each stage's exchange mutual
            # (cosets), so pair/quad/full sums compose correctly under any
            # physical tpb permutation.
            stage_src = [tloc, s2src, s3src]
            for k, d in enumerate([1, 2, 4]):
                rdests = [None] * n_cores
                rdests[d] = (0, d)
                nc.gpsimd.remote_dma_broadcast(
                    out_ap=buf[:, k : k + 1],
                    in_ap=stage_src[k][:],
                    remote_sem=gather_sem,
                    local_sem=send_sem,
                    rdests=rdests,
                    queue_num=k,
                )

            # phase 1: pipelined load + per-chunk reduce; alternate the two
            # HWDGE rings (sync / scalar) so consecutive transfers' fixed
            # costs (~2us completion receipt each) overlap
            with nc.named_scope("load"):
                for i in range(nch):
                    eng = nc.sync if i % 2 == 0 else nc.scalar
                    eng.dma_start(chunks[i][:], slab(inp_d, i))
                    nc.vector.reduce_sum(
                        partials[:, i : i + 1], chunks[i][:], axis=mybir.AxisListType.X
                    )


            # local total, broadcast to all 128 partitions via ones-matmul
            sc_gate = nc.named_scope("gate")
            sc_gate.__enter__()
            plocal = small_pool.tile([P, 1], f32, name="plocal")
            nc.vector.reduce_sum(plocal[:], partials[:], axis=mybir.AxisListType.X)
            tpsum = psum_pool.tile([P, 1], f32, name="tpsum")
            nc.tensor.matmul(tpsum[:], ones[:], plocal[:])
            tlc = nc.vector.tensor_copy(tloc[:], tpsum[:])

            # The preps' deferred RAW binds to each source's writer at PREP
            # trace time (the memset -- measured: without this explicit edge
            # the trigger fires after the memset and broadcasts zeros), so
            # gate each stage's trigger on the REAL value explicitly. Each
            # stage's arrival bumps gather_sem by 2; thresholds accumulate.
            # The combining adds sit alone inside tile_critical blocks
            # (outside one, the schedule-time sim deadlocks on the
            # cross-core semaphore); same-engine (DVE) program order keeps
            # their acc reads race-free.
            gtot = small_pool.tile([P, 1], f32, name="gtot")
            fac = small_pool.tile([P, 1], f32, name="fac")

            trig1 = nc.gpsimd.trigger_dma(count=None, queue_num=0)
            add_dep_helper(
                trig1.ins, tlc.ins, True, "fire stage1 only after real total"
            )
            with tc.tile_critical():
                nc.vector.tensor_tensor(
                    acc2[:], tloc[:], buf[:, 0:1], mybir.AluOpType.add
                )._wait_ge(gather_sem, 2)
            a2c = nc.vector.tensor_copy(s2src[:], acc2[:])
            trig2 = nc.gpsimd.trigger_dma(count=None, queue_num=1)
            add_dep_helper(
                trig2.ins, a2c.ins, True, "fire stage2 only after pair sum"
            )
            with tc.tile_critical():
                nc.vector.tensor_tensor(
                    acc3[:], acc2[:], buf[:, 1:2], mybir.AluOpType.add
                )._wait_ge(gather_sem, 4)
            a3c = nc.vector.tensor_copy(s3src[:], acc3[:])
            trig3 = nc.gpsimd.trigger_dma(count=None, queue_num=2)
            add_dep_helper(
                trig3.ins, a3c.ins, True, "fire stage3 only after quad sum"
            )
            with tc.tile_critical():
                nc.vector.tensor_tensor(
                    gtot[:], acc3[:], buf[:, 2:3], mybir.AluOpType.add
                )._wait_ge(gather_sem, 6)

            # factor = 1[tot>0] * 3*2^63 - 2^63  ->  2**64 or -(2**63) (exact)
            isgt = nc.vector.tensor_scalar(
                fac[:], gtot[:], 0.0, None, mybir.AluOpType.is_gt
            )
            nc.vector.tensor_scalar(
                fac[:],
                fac[:],
                float(3 * 2**63),
                float(-(2**63)),
                mybir.AluOpType.mult,
                mybir.AluOpType.add,
            )

            sc_gate.__exit__(None, None, None)

            # dummy collective for synchronized launch, AFTER the gate so
            # its dispatch parks gpsimd only during the store phase
            dcc_in = dram_pool.tile([P, 1], f32, name="dcc_in")
            dcc_out = dram_pool.tile(
                [n_cores * P, 1], f32, name="dcc_out", addr_space="Shared"
            )
            dcc = nc.gpsimd.collective_compute(
                "AllGather",
                mybir.AluOpType.bypass,
                replica_groups=[list(range(n_cores))],
                ins=[dcc_in.opt()],
                outs=[dcc_out.opt()],
            )
            # add_dep_helper(A, B) = A waits on B -- dcc waits on the gate
            add_dep_helper(
                dcc.ins, isgt.ins, True, "dummy cc after gate resolves"
            )

            # phase 2: in-place scale (DVE) + store, smallest chunk first,
            # alternating HWDGE rings like the loads
            with nc.named_scope("store"):
                for k, i in enumerate(STORE_ORDER):
                    nc.vector.tensor_scalar_mul(chunks[i][:], chunks[i][:], fac[:])
                    eng = nc.sync if k % 2 == 0 else nc.scalar
                    eng.dma_start(slab(out_d, i), chunks[i][:])

    nc.compile()
    return nc


def _run(in_maps, trace=False, trace_cores=None):
    from concourse.bass_utils import run_bass_kernel_spmd

    global _nc
    if _nc is None:
        _nc = _build()
    kw = {}
    if trace_cores is not None:
        kw["trace_cores"] = trace_cores
    return run_bass_kernel_spmd(
        _nc, in_maps, core_ids=list(range(N_CORES)), trace=trace, **kw
    )


def _shard(inp):
    return [
        np.ascontiguousarray(
            inp[c * ROWS_PER_CORE : (c + 1) * ROWS_PER_CORE]
        ).reshape(TOTAL)
        for c in range(N_CORES)
    ]


def _unshard(results):
    out = np.empty((ROWS, 1024, 1024), dtype=np.float32)
    for c in range(N_CORES):
        out[c * ROWS_PER_CORE : (c + 1) * ROWS_PER_CORE] = results[c]["out"].reshape(
            ROWS_PER_CORE, 1024, 1024
        )
    return out


def kernel(**inputs):
    inp = np.ascontiguousarray(np.asarray(inputs["inp"], dtype=np.float32))
    res = _run([{"inp": s} for s in _shard(inp)], trace=False)
    return _unshard(res.results)


def run_traced(inputs, trace_cores=None):
    """Like kernel() but with NTFF profiling; returns (out, res)."""
    inp = np.ascontiguousarray(np.asarray(inputs["inp"], dtype=np.float32))
    res = _run([{"inp": s} for s in _shard(inp)], trace=True, trace_cores=trace_cores)
    return _unshard(res.results), res

